# revision 6
# baseline (speedup 1.0000x reference)
"""Causal self-attention kernel for Trainium2, sharded over 8 NeuronCores. v2.

Problem (hardcoded): x [2, 2048, 1024] fp32, Wq/Wk/Wv/Wo [1024, 1024], bo [1024].
H = 16 heads, head dim 64.

Sharding: tensor-parallel over heads. Each core owns 2 heads (a 128-wide
column slice of Wq/Wk/Wv and the matching 128-row slice of Wo), computes its
partial out-projection y_i = ctx_i @ Wo[rows_i], and the host sums the 8
partials (the "all-reduce") and adds bo.

v2 layout (vs the v1 feature-major design):
  qkT      [128 (2h x 64d), 2 (q|k), T] fp16 from one PSUM->SBUF copy/chunk
  scoresT  [k-tile 128, 2 heads, q-chunk 512] PSUM; K=64 matmuls at base
           partitions 0/64
  expT     exp(scoresT/8), fp16, causal diag masked via tri multiply
  vsb      [k 128, t-tile, (v_h|1)x2] built by DMA-transpose straight from
           vT (no PE transpose, no PSUM drain copy)
  ctx      TOKEN-major [q 128, qt 4, 65] per head: lhsT = ex tile (keys
           contract on partitions), rhs = [v|1]; 65-row outputs halve the PE
           cost vs feature-major, and the softmax denominator lands as a
           per-partition column so normalization is a reciprocal +
           per-partition scalar multiply (no gpsimd broadcast)
  ctxT     per q-tile DMA-transpose of the normalized ctx back to
           feature-major for the out-projection lhsT
  y        out-proj PSUM -> fp16 SBUF copies split across Pool/DVE, DMA
           issued from gpsimd (SWDGE, cheap SEQ)
"""

import sys

import numpy as np

try:
    import concourse.bass as bass  # noqa: F401
except ImportError:  # harness environments without concourse on sys.path
    sys.path.insert(0, "/opt/trn_rl_repo")
    import concourse.bass as bass  # noqa: F401

from contextlib import ExitStack

import concourse.mybir as mybir
import concourse.tile as tile
from concourse import bacc
from concourse.bass import ts
from concourse.bass_utils import run_bass_kernel_spmd

F32 = mybir.dt.float32
F16 = mybir.dt.float16

N_CORES = 8
B, S, E = 2, 2048, 1024
H, D = 16, 64
EL = 128          # local e' width per core (2 heads x 64)
CH = 512          # q-chunk width (one 2-bank PSUM score tile of fp32)
KT = 128          # k-tile width
NE = E // 128     # e-tiles in the contraction dim


def build_attention(batch=B, seq=S, dt_in=F16, dt_out=F16, n_reps=1):
    """Build the per-core Bass program (same program on all 8 cores)."""
    ncb = seq // CH            # q-chunks per batch
    ntt = seq // 128           # t-tiles per batch
    nkt_b = seq // KT          # k-tiles per batch

    nc = bacc.Bacc("TRN2", debug=False, num_devices=N_CORES)

    dt_i = dt_in
    xT = nc.dram_tensor("xT", [E, batch * seq], dt_in, kind="ExternalInput").ap()
    wq = nc.dram_tensor("wq", [128, E], dt_in, kind="ExternalInput").ap()
    wk = nc.dram_tensor("wk", [128, E], dt_in, kind="ExternalInput").ap()
    wv = nc.dram_tensor("wv", [128, E], dt_in, kind="ExternalInput").ap()
    wo = nc.dram_tensor("wo", [EL, E], dt_in, kind="ExternalInput").ap()
    trimask = nc.dram_tensor("trimask", [128, 128], dt_in, kind="ExternalInput").ap()
    onesc = nc.dram_tensor("onesc", [128, 1], dt_in, kind="ExternalInput").ap()
    y = nc.dram_tensor("y", [batch, seq, E], dt_out, kind="ExternalOutput").ap()

    with tile.TileContext(nc) as tc, ExitStack() as ctx, \
            nc.allow_low_precision(reason="fp16 internals validated vs fp64 reference"):
        consts = ctx.enter_context(tc.tile_pool(name="consts", bufs=1))
        xt_pool = ctx.enter_context(tc.tile_pool(name="xt", bufs=NE + 8))
        big = ctx.enter_context(tc.tile_pool(name="big", bufs=2))
        vsb_pool = ctx.enter_context(tc.tile_pool(name="vsb", bufs=2))
        ex_pool = ctx.enter_context(tc.tile_pool(name="ex", bufs=6))
        ctxn_pool = ctx.enter_context(tc.tile_pool(name="ctxn", bufs=4))
        ctxT_pool = ctx.enter_context(tc.tile_pool(name="ctxT", bufs=2))
        ysb_pool = ctx.enter_context(tc.tile_pool(name="ysb", bufs=4))
        small = ctx.enter_context(tc.tile_pool(name="small", bufs=4))
        # PSUM (8 banks): scores 2x2 + ctx 2x1 + shared 1-bank pool
        # (psv/psq/psk/yp) 2x1
        sc_pool = ctx.enter_context(tc.tile_pool(name="scps", bufs=2, space="PSUM"))
        ctx_pool = ctx.enter_context(tc.tile_pool(name="ctxps", bufs=1, space="PSUM"))
        pj_pool = ctx.enter_context(tc.tile_pool(name="pjps", bufs=2, space="PSUM"))

        # ---- constants / weights (resident) ----
        wq_sb = consts.tile([128, E], dt_in)
        wk_sb = consts.tile([128, E], dt_in)
        wv_sb = consts.tile([128, E], dt_in)
        wo_sb = consts.tile([EL, E], dt_in)
        tri_sb = consts.tile([128, 128], dt_in)
        ones_sb = consts.tile([128, 1], dt_in)
        for dst, src in ((wq_sb, wq), (wk_sb, wk), (wv_sb, wv), (wo_sb, wo),
                         (tri_sb, trimask), (ones_sb, onesc)):
            nc.sync.dma_start(dst[:], src)

        rep_cm = tc.For_i(0, n_reps, 1) if n_reps > 1 else None
        if rep_cm is not None:
            rep_cm.__enter__()
        n_y = 0
        # Filler queue: closures of ~one PE matmul each (out-proj steps and
        # next-chunk projection e-steps), drained between each kt's scores
        # and ctx so PE never waits on ACT's exp.
        filler = []
        drain_win = [False]

        def defer_outproj(pb, pqc, pctxT, qts=range(4), hold=None):
            if hold is None:
                hold = {}

            def step(qt, eo):
                def emit():
                    nonlocal n_y
                    yp = pj_pool.tile([128, CH], F32, tag="pj", name="y_ps")
                    nc.tensor.matmul(
                        yp[:], pctxT[:, ts(qt, 128)], wo_sb[:, ts(eo, CH)],
                        start=True, stop=True)
                    if "ysb" not in hold:
                        hold["ysb"] = ysb_pool.tile([128, 4, 2, CH], dt_out,
                                                    tag="ysb", name="ysb")
                    ysb = hold["ysb"]
                    # GPSIMD cannot read PSUM on HW; ACT takes drains only
                    # in exp-idle windows, DVE otherwise
                    if drain_win[0]:
                        nc.scalar.copy(ysb[:, qt, eo, :], yp[:])
                    else:
                        nc.vector.tensor_copy(ysb[:, qt, eo, :], yp[:])
                    n_y += 1
                    if eo == E // CH - 1:
                        # per-t-tile writeback: each partition's row is one
                        # 2KB contiguous DRAM run
                        tt0 = pqc * 4 + qt
                        nc.sync.dma_start(
                            y[pb, tt0 * 128:(tt0 + 1) * 128, :],
                            ysb[:, qt, :, :])
                return emit
            for qt in qts:
                for eo in range(E // CH):
                    filler.append(step(qt, eo))
            return hold

        def enqueue_proj(st, qc):
            """Queue chunk qc's v/q/k projection steps for batch state st."""
            xts_, qkT_, vsb_ = st["xts"], st["qkT"], st["vsb"]
            hold = {}

            def mk_v(tt):
                # token-major v: lhsT = x tile (e contracts), rhs = Wv slice;
                # the output lands as [t, (h d)] = exactly the vsb layout
                def emit():
                    pv = pj_pool.tile([128, 128], F32, tag="pj", name="ps_v")
                    for e in range(NE):
                        nc.tensor.matmul(pv[:], xts_[e][:, ts(tt, 128)],
                                         wv_sb[:, ts(e, 128)],
                                         start=(e == 0), stop=(e == NE - 1))
                    nc.vector.tensor_copy(
                        vsb_[:, tt, 0:130].rearrange(
                            "p (h c) -> p h c", h=2)[:, :, 0:64],
                        pv.rearrange("p (h c) -> p h c", h=2))
                return emit

            def mk(which, e, w_sb, fin):
                def emit():
                    if e == 0:
                        hold[which] = pj_pool.tile([128, CH], F32, tag="pj",
                                                   name=f"ps_{which}")
                    nc.tensor.matmul(hold[which][:], w_sb[:, ts(e, 128)],
                                     xts_[e][:, ts(qc, CH)],
                                     start=(e == 0), stop=(e == NE - 1))
                    if e == NE - 1:
                        fin(hold[which])
                return emit

            def fin_k(ps):
                nc.vector.tensor_copy(qkT_[:, 1, ts(qc, CH)], ps[:])

            def fin_q(ps):
                nc.vector.tensor_copy(qkT_[:, 0, ts(qc, CH)], ps[:])

            # k first: its drain copy gates the next chunk's first scores
            for e in range(NE):
                filler.append(mk("k", e, wk_sb, fin_k))
            for e in range(NE):
                filler.append(mk("q", e, wq_sb, fin_q))
            for tt in range(qc * (CH // 128), (qc + 1) * (CH // 128)):
                filler.append(mk_v(tt))

        def alloc_xts():
            return [xt_pool.tile([128, seq], dt_in, tag="xt", name=f"xt{e}")
                    for e in range(NE)]

        def load_xts_block(xts_, b, qc):
            # one chunk-column block across all e-tiles (8 DMAs)
            for e in range(NE):
                nc.sync.dma_start(
                    xts_[e][:, ts(qc, CH)],
                    xT[e * 128:(e + 1) * 128,
                       b * seq + qc * CH:b * seq + (qc + 1) * CH])

        def new_batch_state(b):
            st = {"b": b, "xts": alloc_xts()}
            st["qkT"] = big.tile([EL, 2, seq], dt_i, tag="qkT", name="qkT")
            st["vsb"] = vsb_pool.tile([128, nkt_b, 130], dt_i, tag="vsb",
                                      name="vsb_all")
            nc.vector.tensor_copy(
                st["vsb"].rearrange("p t (h c) -> p (t h) c", c=65)[:, :, 64:65],
                ones_sb.broadcast_to([128, 2 * nkt_b, 1]))
            return st

        cur = new_batch_state(0)
        for qcc in range(ncb):
            load_xts_block(cur["xts"], 0, qcc)
        nxt = None

        for b in range(batch):
            qkT, vsb_all, xts = cur["qkT"], cur["vsb"], cur["xts"]
            for qc in range(ncb):
                if b == 0 and qc == 0:
                    # rep head: no preceding kt loop to hide this in
                    enqueue_proj(cur, 0)
                    while filler:
                        filler.pop(0)()
                # stage the next batch's x loads (spread across chunks) and
                # the NEXT chunk's projections
                if b + 1 < batch:
                    if qc == 0:
                        nxt = new_batch_state(b + 1)
                    load_xts_block(nxt["xts"], b + 1, qc)
                if qc + 1 < ncb:
                    enqueue_proj(cur, qc + 1)
                elif b + 1 < batch:
                    enqueue_proj(nxt, 0)

                # ---- attention for this chunk, scores one k-tile ahead,
                # filler (out-proj + next-chunk proj) drained per kt ----
                nkt = (qc * CH + CH) // KT  # causal: k-tiles 0 .. nkt-1
                ctxps = [ctx_pool.tile([128, 4, 65], F32, tag=f"ctx{hi}",
                                       name=f"ctx_ps{hi}") for hi in range(2)]
                exs = {}

                def issue_scores(kt):
                    c0 = max(0, kt * KT - qc * CH)
                    n = CH - c0
                    sc = sc_pool.tile([128, 2, CH], F32, tag="sc", name="sc_ps")
                    for hi in range(2):
                        r0 = hi * 64
                        nc.tensor.matmul(
                            sc[:, hi, 0:n],
                            qkT[r0:r0 + 64, 1, ts(kt, KT)],
                            qkT[r0:r0 + 64, 0, qc * CH + c0:(qc + 1) * CH],
                            start=True, stop=True,
                        )
                    ex = ex_pool.tile([128, 2, CH], dt_i, tag="ex", name="ex")
                    nc.scalar.activation(
                        ex[:, :, 0:n], sc[:, :, 0:n],
                        mybir.ActivationFunctionType.Exp, scale=1.0 / np.sqrt(D))
                    if kt * KT >= qc * CH:  # diag tile: causal mask
                        nc.vector.tensor_mul(
                            ex[:, :, 0:128], ex[:, :, 0:128],
                            tri_sb.unsqueeze(1).broadcast_to([128, 2, 128]))
                    exs[kt] = ex

                def issue_ctx(kt):
                    c0 = max(0, kt * KT - qc * CH)
                    qt0 = c0 // 128
                    ex = exs.pop(kt)
                    # PSUM zero-region semantics: start=True only on the FIRST
                    # matmul into each bank; later qt slots' first writes land
                    # on pending-zero bytes and overwrite (no accumulate).
                    for qt in range(qt0, 4):
                        for hi in range(2):
                            nc.tensor.matmul(
                                ctxps[hi][:, qt, :],
                                ex[:, hi, qt * 128 - c0:qt * 128 - c0 + 128],
                                vsb_all[:, kt, hi * 65:(hi + 1) * 65],
                                start=(kt == 0 and qt == 0),
                                stop=(kt == 4 * qc + 3 and qt == 3),
                                skip_group_check=True,
                            )

                last_chunk = b == batch - 1 and qc == ncb - 1
                ctxn = ctxn_pool.tile([128, 4, 2, 64], dt_i, tag="ctxn",
                                      name="ctxn")
                ctxT = ctxT_pool.tile([EL, CH], dt_i, tag="ctxT", name="ctxT")

                def finish_chunk(qts):
                    # normalize (one recip + one mul per head) and transpose
                    # back to feature-major per q-tile
                    q0, q1 = qts[0], qts[-1] + 1
                    nq = q1 - q0
                    for hi in range(2):
                        linv = small.tile([128, 4, 1], F32, tag="linv",
                                          name="linv")
                        nc.vector.reciprocal(linv[:, q0:q1, :],
                                             ctxps[hi][:, q0:q1, 64:65])
                        nc.vector.tensor_mul(
                            ctxn[:, q0:q1, hi, :], ctxps[hi][:, q0:q1, 0:64],
                            linv[:, q0:q1, :].broadcast_to([128, nq, 64]))
                    for qt in qts:
                        nc.sync.dma_start(ctxT[:, ts(qt, 128)],
                                          ctxn[:, qt, :, :], transpose=True)

                issue_scores(0)
                lc_hold = {}
                for kt in range(nkt):
                    if kt + 1 < nkt:
                        issue_scores(kt + 1)
                    for _ in range(2):
                        if filler:
                            filler.pop(0)()
                    issue_ctx(kt)
                    # last chunk: finish q-tiles as their diag k-tile lands
                    # and feed their own out-proj into the filler so the tail
                    # pipelines instead of serializing
                    if last_chunk and 0 <= kt - 4 * qc < 4:
                        finish_chunk([kt - 4 * qc])
                        defer_outproj(b, qc, ctxT, qts=[kt - 4 * qc],
                                      hold=lc_hold)
                if not last_chunk:
                    finish_chunk([0, 1, 2, 3])
                # drain any leftover proj work before the next chunk's scores
                drain_win[0] = True
                while filler:
                    filler.pop(0)()
                drain_win[0] = False
                if not last_chunk:
                    defer_outproj(b, qc, ctxT)

            if nxt is not None:
                cur, nxt = nxt, None

        if rep_cm is not None:
            rep_cm.__exit__(None, None, None)

    nc.compile()
    return nc


def _prep_inputs(x, Wq, Wk, Wv, Wo, dt_in=np.float16):
    """Host-side sharding: transpose x, slice weights per core."""
    batch, seq, _ = x.shape
    xT = np.ascontiguousarray(x.reshape(batch * seq, E).T).astype(dt_in)
    tri = np.triu(np.ones((128, 128), np.float32))  # tri[p, c] = 1 iff p <= c
    ident = np.eye(128, dtype=np.float32)

    def warr(w):  # [E, 128] col-slice -> SBUF layout [128, 8*128]
        return np.ascontiguousarray(
            w.reshape(NE, 128, 128).transpose(1, 0, 2).reshape(128, E)
        ).astype(dt_in)

    in_maps = []
    for i in range(N_CORES):
        cols = slice(i * EL, (i + 1) * EL)
        in_maps.append({
            "xT": xT,
            "wq": warr(Wq[:, cols]),
            "wk": warr(Wk[:, cols]),
            "wv": warr(Wv[:, cols]),
            "wo": np.ascontiguousarray(Wo[cols, :]).astype(dt_in),
            "trimask": tri.astype(dt_in),
            "onesc": np.ones((128, 1), dt_in),
        })
    return in_maps


_CACHE = {}


def _get_nc(batch, seq, dt_in, dt_out):
    key = (batch, seq, dt_in, dt_out)
    if key not in _CACHE:
        _CACHE[key] = build_attention(batch, seq, dt_in, dt_out)
    return _CACHE[key]


DT_IN = F16   # fp16 x/W transfers; projections accumulate fp32 in PSUM
DT_OUT = F16  # fp16 partial-y transfers; host sums in fp32


def kernel(x, Wq, Wk, Wv, Wo, bo, _trace=False):
    x = np.asarray(x, np.float32)
    batch, seq, _ = x.shape
    nc = _get_nc(batch, seq, DT_IN, DT_OUT)
    in_maps = _prep_inputs(x, np.asarray(Wq), np.asarray(Wk), np.asarray(Wv),
                           np.asarray(Wo),
                           dt_in=np.float16 if DT_IN == F16 else np.float32)
    res = run_bass_kernel_spmd(nc, in_maps, core_ids=list(range(N_CORES)),
                               trace=_trace)
    parts = [res.results[i]["y"].astype(np.float32) for i in range(N_CORES)]
    y = np.sum(parts, axis=0, dtype=np.float32) + np.asarray(bo, np.float32)
    if _trace:
        kernel.last_results = res
    return y


# revision 7
# speedup vs baseline: 1.0244x; 1.0244x over previous
"""Causal self-attention kernel for Trainium2, sharded over 8 NeuronCores. v2.

Problem (hardcoded): x [2, 2048, 1024] fp32, Wq/Wk/Wv/Wo [1024, 1024], bo [1024].
H = 16 heads, head dim 64.

Sharding: tensor-parallel over heads. Each core owns 2 heads (a 128-wide
column slice of Wq/Wk/Wv and the matching 128-row slice of Wo), computes its
partial out-projection y_i = ctx_i @ Wo[rows_i], and the host sums the 8
partials (the "all-reduce") and adds bo.

v2 design (vs the v1 feature-major kernel):
  qkT      [128 (2h x 64d), 2 (q|k), T] fp16; q/k PSUM drains on DVE
  scoresT  [k-tile 128, 2 heads, q-chunk 512] PSUM; K=64 matmuls at base
           partitions 0/64, software-pipelined one k-tile ahead of exp
  expT     exp(scoresT/8) on ACT (kept a pure exp stream: no DMA issue or
           copies on the ACT queue mid-loop, avoiding head-of-line blocks)
  v        projected TOKEN-major per t-tile (lhsT = x tile, rhs = Wv) so it
           lands directly in the [k, (v_h|1)x2] ctx-rhs layout -- no
           transpose at all
  ctx      TOKEN-major [q 128, qt 4, 65] per head: lhsT = ex tile, rhs =
           [v|1]; 65-row outputs halve PE cost vs feature-major; the
           softmax denominator lands as a per-partition column so
           normalization is one reciprocal + one broadcast multiply per
           head-chunk.  PSUM zero-region rule: start=True only on the
           first matmul into each bank.
  ctxT     per q-tile SP-issued DMA-transpose back to feature-major for
           the out-projection lhsT
  filler   out-proj + next-chunk projection matmuls run from a queue, two
           per k-tile, so PE stays busy while ACT paces exp; x loads are
           chunk-column blocks and next-batch loads spread across chunks
  y        out-proj PSUM -> fp16 staging on DVE, one contiguous
           per-t-tile writeback DMA issued from SP
"""

import sys

import numpy as np

try:
    import concourse.bass as bass  # noqa: F401
except ImportError:  # harness environments without concourse on sys.path
    sys.path.insert(0, "/opt/trn_rl_repo")
    import concourse.bass as bass  # noqa: F401

from contextlib import ExitStack

import concourse.mybir as mybir
import concourse.tile as tile
from concourse import bacc
from concourse.bass import ts
from concourse.bass_utils import run_bass_kernel_spmd

F32 = mybir.dt.float32
F16 = mybir.dt.float16

N_CORES = 8
B, S, E = 2, 2048, 1024
H, D = 16, 64
EL = 128          # local e' width per core (2 heads x 64)
CH = 512          # q-chunk width (one 2-bank PSUM score tile of fp32)
KT = 128          # k-tile width
NE = E // 128     # e-tiles in the contraction dim


def build_attention(batch=B, seq=S, dt_in=F16, dt_out=F16, n_reps=1):
    """Build the per-core Bass program (same program on all 8 cores)."""
    ncb = seq // CH            # q-chunks per batch
    ntt = seq // 128           # t-tiles per batch
    nkt_b = seq // KT          # k-tiles per batch

    nc = bacc.Bacc("TRN2", debug=False, num_devices=N_CORES)

    dt_i = dt_in
    xT = nc.dram_tensor("xT", [E, batch * seq], dt_in, kind="ExternalInput").ap()
    wq = nc.dram_tensor("wq", [128, E], dt_in, kind="ExternalInput").ap()
    wk = nc.dram_tensor("wk", [128, E], dt_in, kind="ExternalInput").ap()
    wv = nc.dram_tensor("wv", [128, E], dt_in, kind="ExternalInput").ap()
    wo = nc.dram_tensor("wo", [EL, E], dt_in, kind="ExternalInput").ap()
    trimask = nc.dram_tensor("trimask", [128, 128], dt_in, kind="ExternalInput").ap()
    onesc = nc.dram_tensor("onesc", [128, 1], dt_in, kind="ExternalInput").ap()
    y = nc.dram_tensor("y", [batch, seq, E], dt_out, kind="ExternalOutput").ap()

    with tile.TileContext(nc) as tc, ExitStack() as ctx, \
            nc.allow_low_precision(reason="fp16 internals validated vs fp64 reference"):
        consts = ctx.enter_context(tc.tile_pool(name="consts", bufs=1))
        xt_pool = ctx.enter_context(tc.tile_pool(name="xt", bufs=NE + 8))
        big = ctx.enter_context(tc.tile_pool(name="big", bufs=2))
        vsb_pool = ctx.enter_context(tc.tile_pool(name="vsb", bufs=2))
        ex_pool = ctx.enter_context(tc.tile_pool(name="ex", bufs=6))
        ctxn_pool = ctx.enter_context(tc.tile_pool(name="ctxn", bufs=4))
        ctxT_pool = ctx.enter_context(tc.tile_pool(name="ctxT", bufs=2))
        ysb_pool = ctx.enter_context(tc.tile_pool(name="ysb", bufs=4))
        small = ctx.enter_context(tc.tile_pool(name="small", bufs=4))
        # PSUM (8 banks): scores 2x2 + ctx 2x1 + shared 1-bank pool
        # (psv/psq/psk/yp) 2x1
        sc_pool = ctx.enter_context(tc.tile_pool(name="scps", bufs=2, space="PSUM"))
        ctx_pool = ctx.enter_context(tc.tile_pool(name="ctxps", bufs=1, space="PSUM"))
        pj_pool = ctx.enter_context(tc.tile_pool(name="pjps", bufs=2, space="PSUM"))

        # ---- constants / weights (resident) ----
        wq_sb = consts.tile([128, E], dt_in)
        wk_sb = consts.tile([128, E], dt_in)
        wv_sb = consts.tile([128, E], dt_in)
        wo_sb = consts.tile([EL, E], dt_in)
        tri_sb = consts.tile([128, 128], dt_in)
        ones_sb = consts.tile([128, 1], dt_in)
        for dst, src in ((wq_sb, wq), (wk_sb, wk), (wv_sb, wv), (wo_sb, wo),
                         (tri_sb, trimask), (ones_sb, onesc)):
            nc.sync.dma_start(dst[:], src)

        rep_cm = tc.For_i(0, n_reps, 1) if n_reps > 1 else None
        if rep_cm is not None:
            rep_cm.__enter__()
        n_y = 0
        # Filler queue: closures of ~one PE matmul each (out-proj steps and
        # next-chunk projection e-steps), drained between each kt's scores
        # and ctx so PE never waits on ACT's exp.
        filler = []

        def defer_outproj(pb, pqc, pctxT, qts=range(4), hold=None):
            if hold is None:
                hold = {}

            def step(qt, eo):
                def emit():
                    nonlocal n_y
                    yp = pj_pool.tile([128, CH], F32, tag="pj", name="y_ps")
                    nc.tensor.matmul(
                        yp[:], pctxT[:, ts(qt, 128)], wo_sb[:, ts(eo, CH)],
                        start=True, stop=True)
                    if "ysb" not in hold:
                        hold["ysb"] = ysb_pool.tile([128, 4, 2, CH], dt_out,
                                                    tag="ysb", name="ysb")
                    ysb = hold["ysb"]
                    # GPSIMD cannot read PSUM on HW: all drains on DVE
                    nc.vector.tensor_copy(ysb[:, qt, eo, :], yp[:])
                    n_y += 1
                    if eo == E // CH - 1:
                        # per-t-tile writeback: each partition's row is one
                        # 2KB contiguous DRAM run
                        tt0 = pqc * 4 + qt
                        nc.sync.dma_start(
                            y[pb, tt0 * 128:(tt0 + 1) * 128, :],
                            ysb[:, qt, :, :])
                return emit
            for qt in qts:
                for eo in range(E // CH):
                    filler.append(step(qt, eo))
            return hold

        def enqueue_proj(st, qc):
            """Queue chunk qc's v/q/k projection steps for batch state st."""
            xts_, qkT_, vsb_ = st["xts"], st["qkT"], st["vsb"]
            hold = {}

            def mk_v(tt):
                # token-major v: lhsT = x tile (e contracts), rhs = Wv slice;
                # the output lands as [t, (h d)] = exactly the vsb layout
                def emit():
                    pv = pj_pool.tile([128, 128], F32, tag="pj", name="ps_v")
                    for e in range(NE):
                        nc.tensor.matmul(pv[:], xts_[e][:, ts(tt, 128)],
                                         wv_sb[:, ts(e, 128)],
                                         start=(e == 0), stop=(e == NE - 1))
                    nc.vector.tensor_copy(
                        vsb_[:, tt, 0:130].rearrange(
                            "p (h c) -> p h c", h=2)[:, :, 0:64],
                        pv.rearrange("p (h c) -> p h c", h=2))
                return emit

            def mk(which, e, w_sb, fin):
                def emit():
                    if e == 0:
                        hold[which] = pj_pool.tile([128, CH], F32, tag="pj",
                                                   name=f"ps_{which}")
                    nc.tensor.matmul(hold[which][:], w_sb[:, ts(e, 128)],
                                     xts_[e][:, ts(qc, CH)],
                                     start=(e == 0), stop=(e == NE - 1))
                    if e == NE - 1:
                        fin(hold[which])
                return emit

            def fin_k(ps):
                nc.vector.tensor_copy(qkT_[:, 1, ts(qc, CH)], ps[:])

            def fin_q(ps):
                nc.vector.tensor_copy(qkT_[:, 0, ts(qc, CH)], ps[:])

            # k first: its drain copy gates the next chunk's first scores
            for e in range(NE):
                filler.append(mk("k", e, wk_sb, fin_k))
            for e in range(NE):
                filler.append(mk("q", e, wq_sb, fin_q))
            for tt in range(qc * (CH // 128), (qc + 1) * (CH // 128)):
                filler.append(mk_v(tt))

        def alloc_xts():
            return [xt_pool.tile([128, seq], dt_in, tag="xt", name=f"xt{e}")
                    for e in range(NE)]

        def load_xts_block(xts_, b, qc):
            # one chunk-column block across all e-tiles (8 DMAs)
            for e in range(NE):
                nc.sync.dma_start(
                    xts_[e][:, ts(qc, CH)],
                    xT[e * 128:(e + 1) * 128,
                       b * seq + qc * CH:b * seq + (qc + 1) * CH])

        def new_batch_state(b):
            st = {"b": b, "xts": alloc_xts()}
            st["qkT"] = big.tile([EL, 2, seq], dt_i, tag="qkT", name="qkT")
            st["vsb"] = vsb_pool.tile([128, nkt_b, 130], dt_i, tag="vsb",
                                      name="vsb_all")
            nc.vector.tensor_copy(
                st["vsb"].rearrange("p t (h c) -> p (t h) c", c=65)[:, :, 64:65],
                ones_sb.broadcast_to([128, 2 * nkt_b, 1]))
            return st

        cur = new_batch_state(0)
        for qcc in range(ncb):
            load_xts_block(cur["xts"], 0, qcc)
        nxt = None

        for b in range(batch):
            qkT, vsb_all, xts = cur["qkT"], cur["vsb"], cur["xts"]
            for qc in range(ncb):
                if b == 0 and qc == 0:
                    # rep head: no preceding kt loop to hide this in
                    enqueue_proj(cur, 0)
                    while filler:
                        filler.pop(0)()
                # stage the next batch's x loads (spread across chunks) and
                # the NEXT chunk's projections
                if b + 1 < batch:
                    if qc == 0:
                        nxt = new_batch_state(b + 1)
                    load_xts_block(nxt["xts"], b + 1, qc)
                if qc + 1 < ncb:
                    enqueue_proj(cur, qc + 1)
                elif b + 1 < batch:
                    enqueue_proj(nxt, 0)

                # ---- attention for this chunk, scores one k-tile ahead,
                # filler (out-proj + next-chunk proj) drained per kt ----
                nkt = (qc * CH + CH) // KT  # causal: k-tiles 0 .. nkt-1
                ctxps = [ctx_pool.tile([128, 4, 65], F32, tag=f"ctx{hi}",
                                       name=f"ctx_ps{hi}") for hi in range(2)]
                exs = {}

                def issue_scores(kt):
                    c0 = max(0, kt * KT - qc * CH)
                    n = CH - c0
                    sc = sc_pool.tile([128, 2, CH], F32, tag="sc", name="sc_ps")
                    for hi in range(2):
                        r0 = hi * 64
                        nc.tensor.matmul(
                            sc[:, hi, 0:n],
                            qkT[r0:r0 + 64, 1, ts(kt, KT)],
                            qkT[r0:r0 + 64, 0, qc * CH + c0:(qc + 1) * CH],
                            start=True, stop=True,
                        )
                    ex = ex_pool.tile([128, 2, CH], dt_i, tag="ex", name="ex")
                    nc.scalar.activation(
                        ex[:, :, 0:n], sc[:, :, 0:n],
                        mybir.ActivationFunctionType.Exp, scale=1.0 / np.sqrt(D))
                    if kt * KT >= qc * CH:  # diag tile: causal mask
                        nc.vector.tensor_mul(
                            ex[:, :, 0:128], ex[:, :, 0:128],
                            tri_sb.unsqueeze(1).broadcast_to([128, 2, 128]))
                    exs[kt] = ex

                def issue_ctx(kt):
                    c0 = max(0, kt * KT - qc * CH)
                    qt0 = c0 // 128
                    ex = exs.pop(kt)
                    # PSUM zero-region semantics: start=True only on the FIRST
                    # matmul into each bank; later qt slots' first writes land
                    # on pending-zero bytes and overwrite (no accumulate).
                    for qt in range(qt0, 4):
                        for hi in range(2):
                            nc.tensor.matmul(
                                ctxps[hi][:, qt, :],
                                ex[:, hi, qt * 128 - c0:qt * 128 - c0 + 128],
                                vsb_all[:, kt, hi * 65:(hi + 1) * 65],
                                start=(kt == 0 and qt == 0),
                                stop=(kt == 4 * qc + 3 and qt == 3),
                                skip_group_check=True,
                            )

                last_chunk = b == batch - 1 and qc == ncb - 1
                ctxn = ctxn_pool.tile([128, 4, 2, 64], dt_i, tag="ctxn",
                                      name="ctxn")
                ctxT = ctxT_pool.tile([EL, CH], dt_i, tag="ctxT", name="ctxT")

                def finish_chunk(qts):
                    # normalize (one recip + one mul per head) and transpose
                    # back to feature-major per q-tile
                    q0, q1 = qts[0], qts[-1] + 1
                    nq = q1 - q0
                    for hi in range(2):
                        linv = small.tile([128, 4, 1], F32, tag="linv",
                                          name="linv")
                        nc.vector.reciprocal(linv[:, q0:q1, :],
                                             ctxps[hi][:, q0:q1, 64:65])
                        nc.vector.tensor_mul(
                            ctxn[:, q0:q1, hi, :], ctxps[hi][:, q0:q1, 0:64],
                            linv[:, q0:q1, :].broadcast_to([128, nq, 64]))
                    for qt in qts:
                        nc.sync.dma_start(ctxT[:, ts(qt, 128)],
                                          ctxn[:, qt, :, :], transpose=True)

                issue_scores(0)
                lc_hold = {}
                for kt in range(nkt):
                    if kt + 1 < nkt:
                        issue_scores(kt + 1)
                    for _ in range(2):
                        if filler:
                            filler.pop(0)()
                    issue_ctx(kt)
                    # last chunk: finish q-tiles as their diag k-tile lands
                    # and feed their own out-proj into the filler so the tail
                    # pipelines instead of serializing
                    if last_chunk and 0 <= kt - 4 * qc < 4:
                        finish_chunk([kt - 4 * qc])
                        defer_outproj(b, qc, ctxT, qts=[kt - 4 * qc],
                                      hold=lc_hold)
                if not last_chunk:
                    finish_chunk([0, 1, 2, 3])
                # drain any leftover proj work before the next chunk's scores
                while filler:
                    filler.pop(0)()
                if not last_chunk:
                    defer_outproj(b, qc, ctxT)

            if nxt is not None:
                cur, nxt = nxt, None

        if rep_cm is not None:
            rep_cm.__exit__(None, None, None)

    nc.compile()
    return nc


def _prep_inputs(x, Wq, Wk, Wv, Wo, dt_in=np.float16):
    """Host-side sharding: transpose x, slice weights per core."""
    batch, seq, _ = x.shape
    xT = np.ascontiguousarray(x.reshape(batch * seq, E).T).astype(dt_in)
    tri = np.triu(np.ones((128, 128), np.float32))  # tri[p, c] = 1 iff p <= c
    ident = np.eye(128, dtype=np.float32)

    def warr(w):  # [E, 128] col-slice -> SBUF layout [128, 8*128]
        return np.ascontiguousarray(
            w.reshape(NE, 128, 128).transpose(1, 0, 2).reshape(128, E)
        ).astype(dt_in)

    in_maps = []
    for i in range(N_CORES):
        cols = slice(i * EL, (i + 1) * EL)
        in_maps.append({
            "xT": xT,
            "wq": warr(Wq[:, cols]),
            "wk": warr(Wk[:, cols]),
            "wv": warr(Wv[:, cols]),
            "wo": np.ascontiguousarray(Wo[cols, :]).astype(dt_in),
            "trimask": tri.astype(dt_in),
            "onesc": np.ones((128, 1), dt_in),
        })
    return in_maps


_CACHE = {}


def _get_nc(batch, seq, dt_in, dt_out):
    key = (batch, seq, dt_in, dt_out)
    if key not in _CACHE:
        _CACHE[key] = build_attention(batch, seq, dt_in, dt_out)
    return _CACHE[key]


DT_IN = F16   # fp16 x/W transfers; projections accumulate fp32 in PSUM
DT_OUT = F16  # fp16 partial-y transfers; host sums in fp32


def kernel(x, Wq, Wk, Wv, Wo, bo, _trace=False):
    x = np.asarray(x, np.float32)
    batch, seq, _ = x.shape
    nc = _get_nc(batch, seq, DT_IN, DT_OUT)
    in_maps = _prep_inputs(x, np.asarray(Wq), np.asarray(Wk), np.asarray(Wv),
                           np.asarray(Wo),
                           dt_in=np.float16 if DT_IN == F16 else np.float32)
    res = run_bass_kernel_spmd(nc, in_maps, core_ids=list(range(N_CORES)),
                               trace=_trace)
    parts = [res.results[i]["y"].astype(np.float32) for i in range(N_CORES)]
    y = np.sum(parts, axis=0, dtype=np.float32) + np.asarray(bo, np.float32)
    if _trace:
        kernel.last_results = res
    return y
